# revision 46
# baseline (speedup 1.0000x reference)
"""Trainium2 Bass kernel for nn_Attention_51634096833229 (v2.1).

CvT-style conv-projection attention: depthwise 3x3 conv + BN on the 28x28
token image for q/k/v, linear qkv projections, 3-head attention over 785
tokens, output projection.  Data-parallel over batch: B=32 -> 4 samples
(2 sample-pairs) per core on 8 cores.

Design:
  - host supplies PRE-PADDED bf16 images in both alignment parities
    (image at odd / even column offset of 32-wide padded rows), pair-merged:
    chunk0 = channels 0..127 of samples A|B side by side on the free dim,
    chunk1 = channels 128..191 of A (partitions 0-63) and B (64-127).
    cls token stashed at never-read pad position [row 0, col 30].
    -> zero on-device layout prep; every conv tap runs in the DVE 2x mode.
  - depthwise conv + BN entirely on DVE: 27 scalar_tensor_tensor taps per
    chunk-tile, chunk0 processing both samples in one op (FD=1568).
  - K=64 matmuls issued as concurrent row/col-tiled pairs (tile_position
    derived from base partitions): head0+head1 scores, cross-sample head2,
    half-K projection chunks.
  - softmax scale folded into w_q host-side; exp on ACT psum->sbuf bf16;
    psum evacuation copies (qk, PV, v-scatter) on ACT.
  - softmax normalization fused into the output projection: per-head proj
    with K=65 (P^T rows + Z row), one-hot rhs column emits Z token-major at
    psum cols {192,448,704}; one packed DVE reciprocal; 3 scalar_tensor_
    tensor ops combine heads with per-partition 1/Z plus a bias tile.
"""

import sys

sys.path.insert(0, "/opt/trn_rl_repo")

import numpy as np
import ml_dtypes

import concourse.bass as bass
import concourse.mybir as mybir
import concourse.tile as tile
from concourse import bacc
from concourse.bass_utils import run_bass_kernel_spmd

F32 = mybir.dt.float32
BF16 = mybir.dt.bfloat16
AF = mybir.ActivationFunctionType
OP = mybir.AluOpType

B, T, C, CO, NH, D = 32, 785, 192, 192, 3, 64
NCORES = 8
BPC = B // NCORES          # samples per core
NPAIR = BPC // 2           # sample pairs per core
SCALE = float(CO) ** -0.5
BN_EPS = 1e-5
TC = 786                   # y columns: [dummy, cls, img x 784]
FLN = 844                  # flat padded image length (2B-parity copies)
KA = 29                    # image base offset in flat copy A (taps dx in {0,2})
KB = 30                    # image base offset in flat copy B (taps dx == 1)
CLSPOS = 842               # cls position in flat copy A (never read by taps)
NSEG = [(0, 512), (512, T - 512)]
TBLK = [(i * 128, min(128, T - i * 128)) for i in range((T + 127) // 128)]


def build_bass():
    return Kern().build()


class Kern:
    def __init__(self):
        nc = bacc.Bacc(None)
        self.nc = nc
        dd = nc.declare_dram_parameter
        self.xfa0_d = dd("xfa0", [NPAIR, 128, 2 * FLN], BF16, isOutput=False)
        self.xfb0_d = dd("xfb0", [NPAIR, 128, 2 * FLN], BF16, isOutput=False)
        self.xfa1_d = dd("xfa1", [NPAIR, 128, FLN], BF16, isOutput=False)
        self.xfb1_d = dd("xfb1", [NPAIR, 128, FLN], BF16, isOutput=False)
        self.fx0_d = dd("fx0", [NPAIR, 128, 3 * 112], BF16, isOutput=False)
        self.fx1_d = dd("fx1", [NPAIR, 128, 3 * 56], BF16, isOutput=False)
        self.wqk1_d = dd("wqk1", [2, 128, CO], BF16, isOutput=False)
        self.wqk2_d = dd("wqk2", [2, 128, CO], BF16, isOutput=False)
        self.wv1_d = dd("wv1", [128, CO], BF16, isOutput=False)
        self.wv2_d = dd("wv2", [128, CO], BF16, isOutput=False)
        self.wpa_d = dd("wpa", [NH, 65, CO + 1], BF16, isOutput=False)
        self.wc_d = dd("wc", [2, 128, 27], F32, isOutput=False)
        self.bnt_d = dd("bnt", [2, 128, 3], F32, isOutput=False)
        self.bt_d = dd("btile", [128, CO], F32, isOutput=False)
        self.out_d = dd("out", [BPC, T, CO], F32, isOutput=True)

    def build(self):
        nc = self.nc
        from contextlib import ExitStack
        with tile.TileContext(nc) as tc, ExitStack() as es:
            self.consts = es.enter_context(tc.tile_pool(name="consts", bufs=1))
            self.psp = es.enter_context(tc.tile_pool(name="ps", bufs=2, space="PSUM"))
            self.padp = es.enter_context(tc.tile_pool(name="pad", bufs=2))
            self.yp = es.enter_context(tc.tile_pool(name="y", bufs=2))
            self.qkp = es.enter_context(tc.tile_pool(name="qk", bufs=2))
            self.ep = es.enter_context(tc.tile_pool(name="E", bufs=3))
            self.pvp = es.enter_context(tc.tile_pool(name="pv", bufs=2))
            self.rp = es.enter_context(tc.tile_pool(name="r", bufs=3))
            self.tmpp = es.enter_context(tc.tile_pool(name="tmp", bufs=3))
            self.op_ = es.enter_context(tc.tile_pool(name="osb", bufs=2))
            self.vap = es.enter_context(tc.tile_pool(name="vaug", bufs=2))
            self._consts()
            pads = [self._load(pr) for pr in range(NPAIR)]
            # software pipeline: pair p's conv chains (DVE) are emitted
            # before pair p-1's attention so they overlap on different
            # engines; pair p's projection MMs follow the attention.
            st, _ = self._conv_qkv(0, pads[0], interleave=True)
            for pr in range(1, NPAIR):
                _, ycs = self._conv_qkv(pr, pads[pr], interleave=False)
                self._attn_proj(pr - 1, st)
                st = self._qkv_mms(ycs)
            self._attn_proj(NPAIR - 1, st)
        if not nc.is_finalized():
            nc.finalize()
        return nc

    def _consts(self):
        nc, consts = self.nc, self.consts
        self.wqk1, self.wqk2 = [], []
        for i in range(2):
            t1 = consts.tile([128, CO], BF16, tag=f"wqk1{i}", name=f"wqk1{i}")
            nc.sync.dma_start(t1[:], self.wqk1_d[i])
            self.wqk1.append(t1)
            t2 = consts.tile([128, CO], BF16, tag=f"wqk2{i}", name=f"wqk2{i}")
            nc.sync.dma_start(t2[:], self.wqk2_d[i])
            self.wqk2.append(t2)
        self.wv1 = consts.tile([128, CO], BF16, tag="wv1", name="wv1")
        nc.sync.dma_start(self.wv1[:], self.wv1_d[:])
        self.wv2 = consts.tile([128, CO], BF16, tag="wv2", name="wv2")
        nc.sync.dma_start(self.wv2[:], self.wv2_d[:])
        self.wpa = []
        for h in range(NH):
            t = consts.tile([65, CO + 1], BF16, tag=f"wpa{h}", name=f"wpa{h}")
            nc.sync.dma_start(t[:], self.wpa_d[h])
            self.wpa.append(t)
        self.wc, self.bnt = [], []
        for ci in range(2):
            t = consts.tile([128, 27], F32, tag=f"wc{ci}", name=f"wc{ci}")
            nc.sync.dma_start(t[:], self.wc_d[ci])
            self.wc.append(t)
            t2 = consts.tile([128, 3], F32, tag=f"bnt{ci}", name=f"bnt{ci}")
            nc.sync.dma_start(t2[:], self.bnt_d[ci])
            self.bnt.append(t2)
        self.btile = consts.tile([128, CO], F32, tag="btile", name="btile")
        nc.sync.dma_start(self.btile[:], self.bt_d[:])

    def _load(self, pr):
        nc = self.nc
        fa0 = self.padp.tile([128, 2 * FLN], BF16, tag="fa0", name="fa0")
        nc.sync.dma_start(fa0[:], self.xfa0_d[pr])
        fb0 = self.padp.tile([128, 2 * FLN], BF16, tag="fb0", name="fb0")
        nc.sync.dma_start(fb0[:], self.xfb0_d[pr])
        fa1 = self.padp.tile([128, FLN], BF16, tag="fa1", name="fa1")
        nc.sync.dma_start(fa1[:], self.xfa1_d[pr])
        fb1 = self.padp.tile([128, FLN], BF16, tag="fb1", name="fb1")
        nc.sync.dma_start(fb1[:], self.xfb1_d[pr])
        fx0 = self.padp.tile([128, 3 * 112], BF16, tag="fx0", name="fx0")
        nc.sync.dma_start(fx0[:], self.fx0_d[pr])
        fx1 = self.padp.tile([128, 3 * 56], BF16, tag="fx1", name="fx1")
        nc.sync.dma_start(fx1[:], self.fx1_d[pr])
        return (fa0, fb0, fa1, fb1, fx0, fx1)

    def _conv_chain(self, i, j, pads):
        """Depthwise conv i -> y bf16 via flat-1D taps + one fix-column TT.
        j=0: chunk0 of A and B pair-merged on free dim -> y [128, 2*TC];
        j=1: chunk1 (A rows 0-63, B 64-127) -> y [128, TC]."""
        nc = self.nc
        fa0, fb0, fa1, fb1, fx0, fx1 = pads
        if j == 0:
            fa, fb, fx, ci, ns = fa0, fb0, fx0, 0, 2
        else:
            fa, fb, fx, ci, ns = fa1, fb1, fx1, 1, 1
        y = self.yp.tile([128, ns * TC], BF16, tag=f"y{i}{j}", name=f"y{i}{j}")
        yv = y.rearrange("p (s c) -> p s c", s=ns, c=TC)
        yf = yv[:, :, 2:TC]
        fav = fa.rearrange("p (s c) -> p s c", s=ns, c=FLN)
        fbv = fb.rearrange("p (s c) -> p s c", s=ns, c=FLN)
        # cls column first (independent of the tap chain)
        nc.vector.tensor_copy(yv[:, :, 1:2], fav[:, :, CLSPOS:CLSPOS + 1])

        def tapsrc(tap):
            dy, dx = tap // 3, tap % 3
            if dx == 1:
                return fbv[:, :, 2 + 28 * dy:2 + 28 * dy + 784]
            return fav[:, :, 28 * dy + dx:28 * dy + dx + 784]

        if j == 1:
            # chunk1: DVE produces per-tap products (tensor_scalar, fast
            # mode); the idle Pool engine runs the accumulation adds.
            yf2 = y[:, 2:TC]

            def tapsrc2(tap):
                dy, dx = tap // 3, tap % 3
                if dx == 1:
                    return fb[:, 2 + 28 * dy:2 + 28 * dy + 784]
                return fa[:, 28 * dy + dx:28 * dy + dx + 784]

            nc.vector.tensor_scalar(
                yf2, tapsrc2(0), self.wc[ci][:, i * 9:i * 9 + 1],
                self.bnt[ci][:, i:i + 1], OP.mult, OP.add)
            for tap in range(1, 9):
                tmp = self.tmpp.tile([128, 784], BF16, tag="cvt", name="cvt",
                                     bufs=4)
                nc.vector.tensor_scalar(
                    tmp[:], tapsrc2(tap),
                    self.wc[ci][:, i * 9 + tap:i * 9 + tap + 1], 0.0,
                    OP.mult, OP.add)
                nc.gpsimd.tensor_tensor(yf2, yf2, tmp[:], OP.add)
        else:
            for tap in range(9):
                wcol = self.wc[ci][:, i * 9 + tap:i * 9 + tap + 1]
                if tap == 0:
                    nc.vector.tensor_scalar(yf, tapsrc(tap), wcol,
                                            self.bnt[ci][:, i:i + 1],
                                            OP.mult, OP.add)
                else:
                    nc.vector.scalar_tensor_tensor(yf, tapsrc(tap), wcol, yf,
                                                   OP.mult, OP.add)
        # one fix-column TT per sample slot: subtract host-computed wrap
        # garbage at image columns {0, 27}
        for s in range(ns):
            dst = yv[:, s, 2:TC].rearrange(
                "p (a b) -> p a b", a=28, b=28)[:, :, 0:28:27]
            fxs = fx[:, i * ns * 56 + s * 56:i * ns * 56 + (s + 1) * 56]
            nc.vector.tensor_tensor(
                dst, dst, fxs.rearrange("p (a b) -> p a b", a=28, b=2),
                OP.subtract)
        return y

    def _chains_conv(self, i, pads):
        return [self._conv_chain(i, 0, pads), self._conv_chain(i, 1, pads)]

    def _mms_qk(self, i, ysc):
        """Projection matmuls for conv i (q or k). ysc = [y0pair, y1].
        Returns the three qkT tiles [A-heads01, B-heads01, h2-pair]."""
        nc = self.nc
        y0p, y1 = ysc
        ys = [y0p[:, 0:TC], y0p[:, TC:2 * TC], y1]
        row = [None] * 3
        # chunk0 of A and B -> two live psum tiles; half-K matmuls of A
        # (rows 0:64) and B (rows 64:128) emitted adjacently -> concurrent
        pss = [self.psp.tile([128, 1024], F32, tag="mm", name=f"mmq{si}")
               for si in range(2)]
        for si in range(2):
            for (n0, nn) in NSEG:
                nc.tensor.matmul(
                    pss[si][0:128, n0:n0 + nn],
                    self.wqk1[i][:, 0:128],
                    ys[si][:, 1 + n0:1 + n0 + nn],
                    start=True, stop=False)
        for (n0, nn) in NSEG:
            for si in range(2):
                nc.tensor.matmul(
                    pss[si][0:128, n0:n0 + nn],
                    self.wqk2[i][si * 64:(si + 1) * 64, 0:128],
                    ys[2][si * 64:(si + 1) * 64, 1 + n0:1 + n0 + nn],
                    start=False, stop=True)
        for si in range(2):
            dst = self.qkp.tile([128, T], BF16, tag=f"qk{i}{si}",
                                name=f"qk{i}{si}")
            nc.scalar.copy(dst[:], pss[si][0:128, 0:T])
            row[si] = dst
        # head2 of A (tileA rows 0-63, col strips 0-1) and B (tileB rows
        # 64-127, col strips 2-3): col-concurrent, separate psum banks.
        ps2 = [self.psp.tile([128, 1024], F32, tag="mm", name=f"mmh{si}")
               for si in range(2)]
        for (n0, nn) in NSEG:
            nc.tensor.matmul(
                ps2[0][0:64, n0:n0 + nn],
                self.wqk1[i][:, 128:192],
                ys[0][:, 1 + n0:1 + n0 + nn],
                start=True, stop=False)
            nc.tensor.matmul(
                ps2[1][64:128, n0:n0 + nn],
                self.wqk1[i][:, 128:192],
                ys[1][:, 1 + n0:1 + n0 + nn],
                start=True, stop=False)
            nc.tensor.matmul(
                ps2[0][0:64, n0:n0 + nn],
                self.wqk2[i][0:64, 128:192], ys[2][0:64, 1 + n0:1 + n0 + nn],
                start=False, stop=True)
            nc.tensor.matmul(
                ps2[1][64:128, n0:n0 + nn],
                self.wqk2[i][64:128, 128:192], ys[2][64:128, 1 + n0:1 + n0 + nn],
                start=False, stop=True)
        dst = self.qkp.tile([128, T], BF16, tag=f"qk{i}2", name=f"qk{i}2")
        nc.scalar.copy(dst[0:64, :], ps2[0][0:64, 0:T])
        nc.scalar.copy(dst[64:128, :], ps2[1][64:128, 0:T])
        row[2] = dst
        return row

    def _mms_v(self, ysc):
        """v projection (token-major) + vaug scatter. ysc = [y0pair, y1]."""
        nc = self.nc
        y0p, y1 = ysc
        yv = [y0p[:, 0:TC], y0p[:, TC:2 * TC], y1]
        vaug = [
            [self.vap.tile([128, 3 * 65], BF16, tag=f"va{s}{tb}",
                           name=f"va{s}{tb}") for tb in range(len(TBLK))]
            for s in range(2)]
        for s in range(2):
            for tb, (t0, tn) in enumerate(TBLK):
                nc.vector.memset(
                    vaug[s][tb][0:tn, :].rearrange(
                        "p (h d) -> p h d", h=3, d=65)[:, :, 64:65], 1.0)
        for tbq in range(0, len(TBLK), 2):
            pss = [self.psp.tile([128, 1024], F32, tag="mm", name=f"mmv{si}")
                   for si in range(2)]
            for k in range(2):
                if tbq + k >= len(TBLK):
                    break
                t0, tn = TBLK[tbq + k]
                for si in range(2):
                    nc.tensor.matmul(
                        pss[si][0:tn, 512 * k:512 * k + CO],
                        yv[si][:, 1 + t0:1 + t0 + tn],
                        self.wv1[:], start=True, stop=False)
                for si in range(2):
                    nc.tensor.matmul(
                        pss[si][0:tn, 512 * k:512 * k + CO],
                        yv[2][si * 64:(si + 1) * 64, 1 + t0:1 + t0 + tn],
                        self.wv2[si * 64:(si + 1) * 64, :],
                        start=False, stop=True)
            for k in range(2):
                if tbq + k >= len(TBLK):
                    break
                t0, tn = TBLK[tbq + k]
                for si in range(2):
                    dst = vaug[si][tbq + k][0:tn, :].rearrange(
                        "p (h d) -> p h d", h=3, d=65)[:, :, 0:64]
                    nc.scalar.copy(
                        dst,
                        pss[si][0:tn, 512 * k:512 * k + CO].rearrange(
                            "p (h d) -> p h d", h=3, d=64))
        return vaug

    def _conv_qkv(self, pr, pads, interleave):
        """Emit conv chains and projection MMs for a pair.  When interleave
        is True, chains and MMs alternate per conv (fills the PE early);
        otherwise chains only — call _qkv_mms later with the returned state."""
        if interleave:
            qk = [None, None]
            ycs = [None] * 3
            for i in range(2):
                ycs[i] = self._chains_conv(i, pads)
                qk[i] = self._mms_qk(i, ycs[i])
            ycs[2] = self._chains_conv(2, pads)
            vaug = self._mms_v(ycs[2])
            return (qk, vaug), None
        ycs = [self._chains_conv(i, pads) for i in range(3)]
        return None, ycs

    def _qkv_mms(self, ycs):
        qk = [self._mms_qk(i, ycs[i]) for i in range(2)]
        vaug = self._mms_v(ycs[2])
        return qk, vaug

    def _attn_pass(self, lhs_q, lhs_k, va_sel, vaug):
        """One attention pass: two row-tiled units (hh=0 rows 0:64, hh=1 rows
        64:128 of lhs_q/lhs_k).  va_sel[hh] = (si, vaug col base).  Returns
        psum pv tiles [65, T] per hh (caller copies out)."""
        nc = self.nc
        pvt = [self.psp.tile([128, 1024], F32, tag="pv", name="pv")
               for _ in range(2)]
        nblk = len(TBLK)

        def emit_pv(tb, es_):
            t0, tn = TBLK[tb]
            for hh in range(2):
                si, cb = va_sel[hh]
                for (n0, nn) in NSEG:
                    nc.tensor.matmul(
                        pvt[hh][0:65, n0:n0 + nn],
                        vaug[si][tb][0:tn, cb:cb + 65],
                        es_[hh][0:tn, n0:n0 + nn],
                        start=(tb == 0), stop=(tb == nblk - 1))

        prev = None  # PV trails scores by one tb so it never waits on exp
        for tb, (t0, tn) in enumerate(TBLK):
            es_ = []
            for hh in range(2):
                kh = lhs_k[hh * 64:(hh + 1) * 64, :]
                qh = lhs_q[hh * 64:(hh + 1) * 64, :]
                ss = self.psp.tile([128, 1024], F32, tag="mm", name="mm")
                for (n0, nn) in NSEG:
                    nc.tensor.matmul(
                        ss[0:tn, n0:n0 + nn], kh[:, t0:t0 + tn],
                        qh[:, n0:n0 + nn], start=True, stop=True)
                e = self.ep.tile([128, T], BF16, tag="E", name="E")
                nc.scalar.activation(e[0:tn, 0:T], ss[0:tn, 0:T], AF.Exp)
                es_.append(e)
            if prev is not None:
                emit_pv(tb - 1, prev)
            prev = es_
        emit_pv(nblk - 1, prev)
        return pvt

    def _attn_proj(self, pr, st):
        nc = self.nc
        qk, vaug = st
        pvsb = [[None] * NH, [None] * NH]

        def run_pass(which):
            if which == 2:
                pvt = self._attn_pass(qk[0][2], qk[1][2],
                                      [(0, 130), (1, 130)], vaug)
                for si in range(2):
                    dst = self.pvp.tile([65, T], BF16, tag=f"pv{si}2",
                                        name=f"pv{si}2")
                    nc.scalar.copy(dst[:], pvt[si][0:65, 0:T])
                    pvsb[si][2] = dst
            else:
                si = which
                pvt = self._attn_pass(qk[0][si], qk[1][si],
                                      [(si, 0), (si, 65)], vaug)
                for hh in range(2):
                    dst = self.pvp.tile([65, T], BF16, tag=f"pv{si}{hh}",
                                        name=f"pv{si}{hh}")
                    nc.scalar.copy(dst[:], pvt[hh][0:65, 0:T])
                    pvsb[si][hh] = dst

        # order: A-heads01, h2(A+B), proj A, B-heads01, proj B — proj A's
        # DVE work overlaps pass B on the PE.
        run_pass(0)
        run_pass(2)
        self._proj(2 * pr, 0, pvsb)
        run_pass(1)
        self._proj(2 * pr + 1, 1, pvsb)

    def _proj(self, s, si, pvsb):
        # fused projection + softmax normalize + bias
        # psum layout per (s, lc): h0@0 h1@256 h2@512 (Z at 192/448/704)
        nc = self.nc
        if True:
            obuf = self.op_.tile([128, 6 * CO], F32, tag=f"ob{si}", name=f"ob{si}")
            otl = self.op_.tile([17, CO], F32, tag=f"ot{si}", name=f"ot{si}")
            for lc, (l0, ln) in enumerate(TBLK):
                ps = self.psp.tile([128, 1024], F32,
                                   tag=("mm" if lc % 2 == 0 else "pv"),
                                   name="mm")
                for h in range(NH):
                    nc.tensor.matmul(
                        ps[0:ln, 256 * h:256 * h + CO + 1],
                        pvsb[si][h][:, l0:l0 + ln], self.wpa[h][:],
                        start=(h != 1), stop=(h != 0),
                        skip_group_check=True)
                r = self.rp.tile([128, 3], F32, tag="r", name="r")
                nc.vector.reciprocal(
                    r[0:ln, :].rearrange("p (h x) -> p h x", h=3, x=1),
                    ps[0:ln, 0:768].rearrange(
                        "p (h x) -> p h x", h=3, x=256)[:, :, CO:CO + 1])
                tmp = self.tmpp.tile([128, CO], F32, tag="t", name="t")
                nc.vector.scalar_tensor_tensor(
                    tmp[0:ln, :], ps[0:ln, 0:CO], r[0:ln, 0:1],
                    self.btile[0:ln, :], OP.mult, OP.add)
                nc.vector.scalar_tensor_tensor(
                    tmp[0:ln, :], ps[0:ln, 256:256 + CO], r[0:ln, 1:2],
                    tmp[0:ln, :], OP.mult, OP.add)
                dst = obuf[:, lc * CO:(lc + 1) * CO] if lc < 6 else otl[:]
                nc.vector.scalar_tensor_tensor(
                    dst[0:ln, :], ps[0:ln, 512:512 + CO], r[0:ln, 2:3],
                    tmp[0:ln, :], OP.mult, OP.add)
            nc.sync.dma_start(
                self.out_d[s, 0:768, :].rearrange("(n p) c -> p n c", p=128),
                obuf[:].rearrange("p (n c) -> p n c", n=6, c=CO))
            nc.sync.dma_start(self.out_d[s, 768:785, :], otl[:])


_NC_CACHE = None


def _flat_parity(xi, k):
    """xi [n, 784] -> [n, FLN] flat padded image at offset k."""
    n = xi.shape[0]
    p = np.zeros((n, FLN), dtype=ml_dtypes.bfloat16)
    p[:, k:k + 784] = xi
    return p


def _prep_host(inputs):
    x = np.asarray(inputs["x"], dtype=np.float32)
    conv_w = np.asarray(inputs["conv_w"], dtype=np.float32)   # [3,C,1,3,3]
    bn_scale = np.asarray(inputs["bn_scale"], dtype=np.float32)
    bn_bias = np.asarray(inputs["bn_bias"], dtype=np.float32)
    bn_mean = np.asarray(inputs["bn_mean"], dtype=np.float32)
    bn_var = np.asarray(inputs["bn_var"], dtype=np.float32)
    w_qkv = np.asarray(inputs["w_qkv"], dtype=np.float32)     # [3,CO,C]
    w_proj = np.asarray(inputs["w_proj"], dtype=np.float32)   # [CO,CO]
    b_proj = np.asarray(inputs["b_proj"], dtype=np.float32)   # [CO]

    xt = x.transpose(0, 2, 1).astype(ml_dtypes.bfloat16)       # [B, C, T]
    xcls = xt[:, :, 0]                                         # [B, C]
    ximg = xt[:, :, 1:]                                        # [B, C, 784]
    fa = np.zeros((B, C, FLN), dtype=ml_dtypes.bfloat16)
    fb = np.zeros((B, C, FLN), dtype=ml_dtypes.bfloat16)
    for b in range(B):
        fa[b] = _flat_parity(ximg[b], KA)
        fb[b] = _flat_parity(ximg[b], KB)
    fa[:, :, CLSPOS] = xcls
    # chunk0 pair-merged on free dim: [NP, 128, 2*FLN] = A | B
    xfa0 = np.concatenate([fa[0::2, 0:128], fa[1::2, 0:128]], axis=2)
    xfb0 = np.concatenate([fb[0::2, 0:128], fb[1::2, 0:128]], axis=2)
    # chunk1: A ch128.. on partitions 0-63, B on 64-127
    xfa1 = np.concatenate([fa[0::2, 128:192], fa[1::2, 128:192]], axis=1)
    xfb1 = np.concatenate([fb[0::2, 128:192], fb[1::2, 128:192]], axis=1)

    # BN fold into taps
    s = bn_scale / np.sqrt(bn_var + BN_EPS)                    # [3,C]
    wtap = conv_w[:, :, 0, :, :].reshape(3, C, 9) * s[:, :, None]
    bterm = bn_bias - bn_mean * s                               # [3,C]
    wc_full = np.ascontiguousarray(wtap.transpose(1, 0, 2).reshape(C, 27))
    bnt_full = np.ascontiguousarray(bterm.T)                   # [C,3]
    dup = lambda a: np.concatenate([a[128:192], a[128:192]], 0)
    wc_h = np.stack([wc_full[0:128], dup(wc_full)]).astype(np.float32)
    bnt_h = np.stack([bnt_full[0:128], dup(bnt_full)]).astype(np.float32)

    # wrap-garbage fix columns: fix[i, b, c, y, 0] = sum_dy w*img[y+dy-2, 27]
    # (left, x=0), [..., 1] = sum_dy w*img[y+dy, 0] (right, x=27)
    img3 = ximg.astype(np.float32).reshape(B, C, 28, 28)
    fix = np.zeros((3, B, C, 28, 2), dtype=np.float32)
    for i in range(3):
        for dy in range(3):
            wl = wtap[i, :, dy * 3]
            wr = wtap[i, :, dy * 3 + 2]
            for y in range(28):
                r = y + dy - 2
                if 0 <= r < 28:
                    fix[i, :, :, y, 0] += wl[None, :] * img3[:, :, r, 27]
                r2 = y + dy
                if 0 <= r2 < 28:
                    fix[i, :, :, y, 1] += wr[None, :] * img3[:, :, r2, 0]
    fix = fix.astype(ml_dtypes.bfloat16)
    fixf = fix.reshape(3, B, C, 56)
    NP = B // 2
    # fx0 [NP, 128, 3*112]: per conv i: A fixes (56) then B fixes (56)
    fx0 = np.concatenate([fixf[:, 0::2, 0:128], fixf[:, 1::2, 0:128]],
                         axis=3)                      # [3, NP, 128, 112]
    fx0 = np.ascontiguousarray(
        fx0.transpose(1, 2, 0, 3).reshape(NP, 128, 3 * 112))
    # fx1 [NP, 128, 3*56]: chunk1, A rows 0-63 / B rows 64-127
    fx1 = np.concatenate([fixf[:, 0::2, 128:192], fixf[:, 1::2, 128:192]],
                         axis=2)                      # [3, NP, 128, 56]
    fx1 = np.ascontiguousarray(
        fx1.transpose(1, 2, 0, 3).reshape(NP, 128, 3 * 56))

    # q/k projection weights as lhsT [c, o]; fold softmax scale into q
    wq = w_qkv[0].T * SCALE                                    # [C, CO]
    wk = w_qkv[1].T
    wv = w_qkv[2].T
    wqk1_h = np.stack([wq[0:128], wk[0:128]]).astype(ml_dtypes.bfloat16)
    wqk2_h = np.stack([dup(wq), dup(wk)]).astype(ml_dtypes.bfloat16)
    wv1_h = wv[0:128].astype(ml_dtypes.bfloat16)
    wv2_h = dup(wv).astype(ml_dtypes.bfloat16)

    # per-head output projection rhs [65, 193]: rows 0-63 = Wp_h^T, row 64 =
    # one-hot at col 192 (emits Z token-major)
    wpa_h = np.zeros((NH, 65, CO + 1), dtype=ml_dtypes.bfloat16)
    for h in range(NH):
        wpa_h[h, 0:64, 0:CO] = w_proj[:, h * 64:(h + 1) * 64].T.astype(
            ml_dtypes.bfloat16)
        wpa_h[h, 64, CO] = 1.0
    btile_h = np.ascontiguousarray(
        np.broadcast_to(b_proj[None, :], (128, CO))).astype(np.float32)

    return (xfa0, xfb0, xfa1, xfb1, fx0, fx1, wqk1_h, wqk2_h, wv1_h, wv2_h,
            wpa_h, wc_h, bnt_h, btile_h)


def kernel(**inputs):
    global _NC_CACHE
    (xfa0, xfb0, xfa1, xfb1, fx0, fx1, wqk1_h, wqk2_h, wv1_h, wv2_h,
     wpa_h, wc_h, bnt_h, btile_h) = _prep_host(inputs)

    if _NC_CACHE is None:
        _NC_CACHE = build_bass()
    nc = _NC_CACHE

    PPC = NPAIR  # pairs per core
    sh = lambda a: a.reshape(NCORES, PPC, *a.shape[1:])
    xfa0, xfb0, xfa1, xfb1 = sh(xfa0), sh(xfb0), sh(xfa1), sh(xfb1)
    fx0, fx1 = sh(fx0), sh(fx1)
    in_maps = [
        {"xfa0": np.ascontiguousarray(xfa0[c]),
         "xfb0": np.ascontiguousarray(xfb0[c]),
         "xfa1": np.ascontiguousarray(xfa1[c]),
         "xfb1": np.ascontiguousarray(xfb1[c]),
         "fx0": np.ascontiguousarray(fx0[c]),
         "fx1": np.ascontiguousarray(fx1[c]),
         "wqk1": wqk1_h, "wqk2": wqk2_h, "wv1": wv1_h, "wv2": wv2_h,
         "wpa": wpa_h, "wc": wc_h, "bnt": bnt_h,
         "btile": btile_h}
        for c in range(NCORES)
    ]
    res = run_bass_kernel_spmd(nc, in_maps, list(range(NCORES)), **RUN_KWARGS)
    global LAST_RESULTS
    LAST_RESULTS = res
    out = np.concatenate([np.asarray(r["out"]) for r in res.results], axis=0)
    return out.reshape(B, T, CO).astype(np.float32)


RUN_KWARGS = {}
LAST_RESULTS = None


# revision 47
# speedup vs baseline: 1.1041x; 1.1041x over previous
"""Trainium2 Bass kernel for nn_Attention_51634096833229 (v2.1).

CvT-style conv-projection attention: depthwise 3x3 conv + BN on the 28x28
token image for q/k/v, linear qkv projections, 3-head attention over 785
tokens, output projection.  Data-parallel over batch: B=32 -> 4 samples
(2 sample-pairs) per core on 8 cores.

Design:
  - host supplies PRE-PADDED bf16 images in both alignment parities
    (image at odd / even column offset of 32-wide padded rows), pair-merged:
    chunk0 = channels 0..127 of samples A|B side by side on the free dim,
    chunk1 = channels 128..191 of A (partitions 0-63) and B (64-127).
    cls token stashed at never-read pad position [row 0, col 30].
    -> zero on-device layout prep; every conv tap runs in the DVE 2x mode.
  - depthwise conv + BN entirely on DVE: 27 scalar_tensor_tensor taps per
    chunk-tile, chunk0 processing both samples in one op (FD=1568).
  - K=64 matmuls issued as concurrent row/col-tiled pairs (tile_position
    derived from base partitions): head0+head1 scores, cross-sample head2,
    half-K projection chunks.
  - softmax scale folded into w_q host-side; exp on ACT psum->sbuf bf16;
    psum evacuation copies (qk, PV, v-scatter) on ACT.
  - softmax normalization fused into the output projection: per-head proj
    with K=65 (P^T rows + Z row), one-hot rhs column emits Z token-major at
    psum cols {192,448,704}; one packed DVE reciprocal; 3 scalar_tensor_
    tensor ops combine heads with per-partition 1/Z plus a bias tile.
"""

import sys

sys.path.insert(0, "/opt/trn_rl_repo")

import numpy as np
import ml_dtypes

import concourse.bass as bass
import concourse.mybir as mybir
import concourse.tile as tile
from concourse import bacc
from concourse.bass_utils import run_bass_kernel_spmd

F32 = mybir.dt.float32
BF16 = mybir.dt.bfloat16
AF = mybir.ActivationFunctionType
OP = mybir.AluOpType

B, T, C, CO, NH, D = 32, 785, 192, 192, 3, 64
NCORES = 8
BPC = B // NCORES          # samples per core
NPAIR = BPC // 2           # sample pairs per core
SCALE = float(CO) ** -0.5
BN_EPS = 1e-5
TC = 786                   # y columns: [dummy, cls, img x 784]
FLN = 844                  # flat padded image length (2B-parity copies)
KA = 29                    # image base offset in flat copy A (taps dx in {0,2})
KB = 30                    # image base offset in flat copy B (taps dx == 1)
CLSPOS = 842               # cls position in flat copy A (never read by taps)
NSEG = [(0, 512), (512, T - 512)]
TBLK = [(i * 128, min(128, T - i * 128)) for i in range((T + 127) // 128)]


def build_bass():
    return Kern().build()


class Kern:
    def __init__(self):
        nc = bacc.Bacc(None)
        self.nc = nc
        dd = nc.declare_dram_parameter
        self.xfa0_d = dd("xfa0", [NPAIR, 128, 2 * FLN], BF16, isOutput=False)
        self.xfb0_d = dd("xfb0", [NPAIR, 128, 2 * FLN], BF16, isOutput=False)
        self.xfa1_d = dd("xfa1", [NPAIR, 128, FLN], BF16, isOutput=False)
        self.xfb1_d = dd("xfb1", [NPAIR, 128, FLN], BF16, isOutput=False)
        self.fx0_d = dd("fx0", [NPAIR, 128, 3 * 112], BF16, isOutput=False)
        self.fx1_d = dd("fx1", [NPAIR, 128, 3 * 56], BF16, isOutput=False)
        self.wqk1_d = dd("wqk1", [2, 128, CO], BF16, isOutput=False)
        self.wqk2_d = dd("wqk2", [2, 128, CO], BF16, isOutput=False)
        self.wv1_d = dd("wv1", [128, CO], BF16, isOutput=False)
        self.wv2_d = dd("wv2", [128, CO], BF16, isOutput=False)
        self.wpa_d = dd("wpa", [NH, 65, CO + 1], BF16, isOutput=False)
        self.wc_d = dd("wc", [2, 128, 27], F32, isOutput=False)
        self.bnt_d = dd("bnt", [2, 128, 3], F32, isOutput=False)
        self.bt_d = dd("btile", [128, CO], F32, isOutput=False)
        self.out_d = dd("out", [BPC, T, CO], F32, isOutput=True)

    def build(self):
        nc = self.nc
        from contextlib import ExitStack
        with tile.TileContext(nc) as tc, ExitStack() as es:
            self.consts = es.enter_context(tc.tile_pool(name="consts", bufs=1))
            self.psp = es.enter_context(tc.tile_pool(name="ps", bufs=2, space="PSUM"))
            self.padp = es.enter_context(tc.tile_pool(name="pad", bufs=2))
            self.yp = es.enter_context(tc.tile_pool(name="y", bufs=2))
            self.qkp = es.enter_context(tc.tile_pool(name="qk", bufs=2))
            self.ep = es.enter_context(tc.tile_pool(name="E", bufs=3))
            self.pvp = es.enter_context(tc.tile_pool(name="pv", bufs=2))
            self.rp = es.enter_context(tc.tile_pool(name="r", bufs=3))
            self.tmpp = es.enter_context(tc.tile_pool(name="tmp", bufs=3))
            self.op_ = es.enter_context(tc.tile_pool(name="osb", bufs=2))
            self.vap = es.enter_context(tc.tile_pool(name="vaug", bufs=2))
            self._consts()
            pads = [self._load(pr) for pr in range(NPAIR)]
            # software pipeline: pair p's conv chains (DVE) are emitted
            # before pair p-1's attention so they overlap on different
            # engines; pair p's projection MMs follow the attention.
            st, _ = self._conv_qkv(0, pads[0], interleave=True)
            for pr in range(1, NPAIR):
                _, ycs = self._conv_qkv(pr, pads[pr], interleave=False)
                self._attn_proj(pr - 1, st)
                st = self._qkv_mms(ycs)
            self._attn_proj(NPAIR - 1, st)
        if not nc.is_finalized():
            nc.finalize()
        return nc

    def _consts(self):
        nc, consts = self.nc, self.consts
        self.wqk1, self.wqk2 = [], []
        for i in range(2):
            t1 = consts.tile([128, CO], BF16, tag=f"wqk1{i}", name=f"wqk1{i}")
            nc.sync.dma_start(t1[:], self.wqk1_d[i])
            self.wqk1.append(t1)
            t2 = consts.tile([128, CO], BF16, tag=f"wqk2{i}", name=f"wqk2{i}")
            nc.sync.dma_start(t2[:], self.wqk2_d[i])
            self.wqk2.append(t2)
        self.wv1 = consts.tile([128, CO], BF16, tag="wv1", name="wv1")
        nc.sync.dma_start(self.wv1[:], self.wv1_d[:])
        self.wv2 = consts.tile([128, CO], BF16, tag="wv2", name="wv2")
        nc.sync.dma_start(self.wv2[:], self.wv2_d[:])
        self.wpa = []
        for h in range(NH):
            t = consts.tile([65, CO + 1], BF16, tag=f"wpa{h}", name=f"wpa{h}")
            nc.sync.dma_start(t[:], self.wpa_d[h])
            self.wpa.append(t)
        self.wc, self.bnt = [], []
        for ci in range(2):
            t = consts.tile([128, 27], F32, tag=f"wc{ci}", name=f"wc{ci}")
            nc.sync.dma_start(t[:], self.wc_d[ci])
            self.wc.append(t)
            t2 = consts.tile([128, 3], F32, tag=f"bnt{ci}", name=f"bnt{ci}")
            nc.sync.dma_start(t2[:], self.bnt_d[ci])
            self.bnt.append(t2)
        self.btile = consts.tile([128, CO], F32, tag="btile", name="btile")
        nc.sync.dma_start(self.btile[:], self.bt_d[:])

    def _load(self, pr):
        nc = self.nc
        fa0 = self.padp.tile([128, 2 * FLN], BF16, tag="fa0", name="fa0")
        nc.sync.dma_start(fa0[:], self.xfa0_d[pr])
        fb0 = self.padp.tile([128, 2 * FLN], BF16, tag="fb0", name="fb0")
        nc.sync.dma_start(fb0[:], self.xfb0_d[pr])
        fa1 = self.padp.tile([128, FLN], BF16, tag="fa1", name="fa1")
        nc.sync.dma_start(fa1[:], self.xfa1_d[pr])
        fb1 = self.padp.tile([128, FLN], BF16, tag="fb1", name="fb1")
        nc.sync.dma_start(fb1[:], self.xfb1_d[pr])
        fx0 = self.padp.tile([128, 3 * 112], BF16, tag="fx0", name="fx0")
        nc.sync.dma_start(fx0[:], self.fx0_d[pr])
        fx1 = self.padp.tile([128, 3 * 56], BF16, tag="fx1", name="fx1")
        nc.sync.dma_start(fx1[:], self.fx1_d[pr])
        return (fa0, fb0, fa1, fb1, fx0, fx1)

    def _conv_chain(self, i, j, pads):
        """Depthwise conv i -> y bf16 via flat-1D taps + one fix-column TT.
        j=0: chunk0 of A and B pair-merged on free dim -> y [128, 2*TC];
        j=1: chunk1 (A rows 0-63, B 64-127) -> y [128, TC]."""
        nc = self.nc
        fa0, fb0, fa1, fb1, fx0, fx1 = pads
        if j == 0:
            fa, fb, fx, ci, ns = fa0, fb0, fx0, 0, 2
        else:
            fa, fb, fx, ci, ns = fa1, fb1, fx1, 1, 1
        y = self.yp.tile([128, ns * TC], BF16, tag=f"y{i}{j}", name=f"y{i}{j}")
        yv = y.rearrange("p (s c) -> p s c", s=ns, c=TC)
        yf = yv[:, :, 2:TC]
        fav = fa.rearrange("p (s c) -> p s c", s=ns, c=FLN)
        fbv = fb.rearrange("p (s c) -> p s c", s=ns, c=FLN)
        # cls column first (independent of the tap chain)
        nc.vector.tensor_copy(yv[:, :, 1:2], fav[:, :, CLSPOS:CLSPOS + 1])

        def tapsrc(tap):
            dy, dx = tap // 3, tap % 3
            if dx == 1:
                return fbv[:, :, 2 + 28 * dy:2 + 28 * dy + 784]
            return fav[:, :, 28 * dy + dx:28 * dy + dx + 784]

        if False:
            pass
        else:
            for tap in range(9):
                wcol = self.wc[ci][:, i * 9 + tap:i * 9 + tap + 1]
                if tap == 0:
                    nc.vector.tensor_scalar(yf, tapsrc(tap), wcol,
                                            self.bnt[ci][:, i:i + 1],
                                            OP.mult, OP.add)
                else:
                    nc.vector.scalar_tensor_tensor(yf, tapsrc(tap), wcol, yf,
                                                   OP.mult, OP.add)
        # one fix-column TT per sample slot: subtract host-computed wrap
        # garbage at image columns {0, 27}
        for s in range(ns):
            dst = yv[:, s, 2:TC].rearrange(
                "p (a b) -> p a b", a=28, b=28)[:, :, 0:28:27]
            fxs = fx[:, i * ns * 56 + s * 56:i * ns * 56 + (s + 1) * 56]
            nc.vector.tensor_tensor(
                dst, dst, fxs.rearrange("p (a b) -> p a b", a=28, b=2),
                OP.subtract)
        return y

    def _chains_conv(self, i, pads):
        return [self._conv_chain(i, 0, pads), self._conv_chain(i, 1, pads)]

    def _mms_qk(self, i, ysc):
        """Projection matmuls for conv i (q or k). ysc = [y0pair, y1].
        Returns the three qkT tiles [A-heads01, B-heads01, h2-pair]."""
        nc = self.nc
        y0p, y1 = ysc
        ys = [y0p[:, 0:TC], y0p[:, TC:2 * TC], y1]
        row = [None] * 3
        # chunk0 of A and B -> two live psum tiles; half-K matmuls of A
        # (rows 0:64) and B (rows 64:128) emitted adjacently -> concurrent
        pss = [self.psp.tile([128, 1024], F32, tag="mm", name=f"mmq{si}")
               for si in range(2)]
        for si in range(2):
            for (n0, nn) in NSEG:
                nc.tensor.matmul(
                    pss[si][0:128, n0:n0 + nn],
                    self.wqk1[i][:, 0:128],
                    ys[si][:, 1 + n0:1 + n0 + nn],
                    start=True, stop=False)
        for (n0, nn) in NSEG:
            for si in range(2):
                nc.tensor.matmul(
                    pss[si][0:128, n0:n0 + nn],
                    self.wqk2[i][si * 64:(si + 1) * 64, 0:128],
                    ys[2][si * 64:(si + 1) * 64, 1 + n0:1 + n0 + nn],
                    start=False, stop=True)
        for si in range(2):
            dst = self.qkp.tile([128, T], BF16, tag=f"qk{i}{si}",
                                name=f"qk{i}{si}")
            nc.scalar.copy(dst[:], pss[si][0:128, 0:T])
            row[si] = dst
        # head2 of A (tileA rows 0-63, col strips 0-1) and B (tileB rows
        # 64-127, col strips 2-3): col-concurrent, separate psum banks.
        ps2 = [self.psp.tile([128, 1024], F32, tag="mm", name=f"mmh{si}")
               for si in range(2)]
        for (n0, nn) in NSEG:
            nc.tensor.matmul(
                ps2[0][0:64, n0:n0 + nn],
                self.wqk1[i][:, 128:192],
                ys[0][:, 1 + n0:1 + n0 + nn],
                start=True, stop=False)
            nc.tensor.matmul(
                ps2[1][64:128, n0:n0 + nn],
                self.wqk1[i][:, 128:192],
                ys[1][:, 1 + n0:1 + n0 + nn],
                start=True, stop=False)
            nc.tensor.matmul(
                ps2[0][0:64, n0:n0 + nn],
                self.wqk2[i][0:64, 128:192], ys[2][0:64, 1 + n0:1 + n0 + nn],
                start=False, stop=True)
            nc.tensor.matmul(
                ps2[1][64:128, n0:n0 + nn],
                self.wqk2[i][64:128, 128:192], ys[2][64:128, 1 + n0:1 + n0 + nn],
                start=False, stop=True)
        dst = self.qkp.tile([128, T], BF16, tag=f"qk{i}2", name=f"qk{i}2")
        nc.scalar.copy(dst[0:64, :], ps2[0][0:64, 0:T])
        nc.scalar.copy(dst[64:128, :], ps2[1][64:128, 0:T])
        row[2] = dst
        return row

    def _mms_v(self, ysc):
        """v projection (token-major) + vaug scatter. ysc = [y0pair, y1]."""
        nc = self.nc
        y0p, y1 = ysc
        yv = [y0p[:, 0:TC], y0p[:, TC:2 * TC], y1]
        vaug = [
            [self.vap.tile([128, 3 * 65], BF16, tag=f"va{s}{tb}",
                           name=f"va{s}{tb}") for tb in range(len(TBLK))]
            for s in range(2)]
        for s in range(2):
            for tb, (t0, tn) in enumerate(TBLK):
                nc.vector.memset(
                    vaug[s][tb][0:tn, :].rearrange(
                        "p (h d) -> p h d", h=3, d=65)[:, :, 64:65], 1.0)
        for tbq in range(0, len(TBLK), 2):
            pss = [self.psp.tile([128, 1024], F32, tag="mm", name=f"mmv{si}")
                   for si in range(2)]
            for k in range(2):
                if tbq + k >= len(TBLK):
                    break
                t0, tn = TBLK[tbq + k]
                for si in range(2):
                    nc.tensor.matmul(
                        pss[si][0:tn, 512 * k:512 * k + CO],
                        yv[si][:, 1 + t0:1 + t0 + tn],
                        self.wv1[:], start=True, stop=False)
                for si in range(2):
                    nc.tensor.matmul(
                        pss[si][0:tn, 512 * k:512 * k + CO],
                        yv[2][si * 64:(si + 1) * 64, 1 + t0:1 + t0 + tn],
                        self.wv2[si * 64:(si + 1) * 64, :],
                        start=False, stop=True)
            for k in range(2):
                if tbq + k >= len(TBLK):
                    break
                t0, tn = TBLK[tbq + k]
                for si in range(2):
                    dst = vaug[si][tbq + k][0:tn, :].rearrange(
                        "p (h d) -> p h d", h=3, d=65)[:, :, 0:64]
                    nc.scalar.copy(
                        dst,
                        pss[si][0:tn, 512 * k:512 * k + CO].rearrange(
                            "p (h d) -> p h d", h=3, d=64))
        return vaug

    def _conv_qkv(self, pr, pads, interleave):
        """Emit conv chains and projection MMs for a pair.  When interleave
        is True, chains and MMs alternate per conv (fills the PE early);
        otherwise chains only — call _qkv_mms later with the returned state."""
        if interleave:
            qk = [None, None]
            ycs = [None] * 3
            for i in range(2):
                ycs[i] = self._chains_conv(i, pads)
                qk[i] = self._mms_qk(i, ycs[i])
            ycs[2] = self._chains_conv(2, pads)
            vaug = self._mms_v(ycs[2])
            return (qk, vaug), None
        ycs = [self._chains_conv(i, pads) for i in range(3)]
        return None, ycs

    def _qkv_mms(self, ycs):
        qk = [self._mms_qk(i, ycs[i]) for i in range(2)]
        vaug = self._mms_v(ycs[2])
        return qk, vaug

    def _attn_pass(self, lhs_q, lhs_k, va_sel, vaug):
        """One attention pass: two row-tiled units (hh=0 rows 0:64, hh=1 rows
        64:128 of lhs_q/lhs_k).  va_sel[hh] = (si, vaug col base).  Returns
        psum pv tiles [65, T] per hh (caller copies out)."""
        nc = self.nc
        pvt = [self.psp.tile([128, 1024], F32, tag="pv", name="pv")
               for _ in range(2)]
        nblk = len(TBLK)

        def emit_pv(tb, es_):
            t0, tn = TBLK[tb]
            for hh in range(2):
                si, cb = va_sel[hh]
                for (n0, nn) in NSEG:
                    nc.tensor.matmul(
                        pvt[hh][0:65, n0:n0 + nn],
                        vaug[si][tb][0:tn, cb:cb + 65],
                        es_[hh][0:tn, n0:n0 + nn],
                        start=(tb == 0), stop=(tb == nblk - 1))

        prev = None  # PV trails scores by one tb so it never waits on exp
        for tb, (t0, tn) in enumerate(TBLK):
            es_ = []
            for hh in range(2):
                kh = lhs_k[hh * 64:(hh + 1) * 64, :]
                qh = lhs_q[hh * 64:(hh + 1) * 64, :]
                ss = self.psp.tile([128, 1024], F32, tag="mm", name="mm")
                for (n0, nn) in NSEG:
                    nc.tensor.matmul(
                        ss[0:tn, n0:n0 + nn], kh[:, t0:t0 + tn],
                        qh[:, n0:n0 + nn], start=True, stop=True)
                e = self.ep.tile([128, T], BF16, tag="E", name="E")
                nc.scalar.activation(e[0:tn, 0:T], ss[0:tn, 0:T], AF.Exp)
                es_.append(e)
            if prev is not None:
                emit_pv(tb - 1, prev)
            prev = es_
        emit_pv(nblk - 1, prev)
        return pvt

    def _attn_proj(self, pr, st):
        nc = self.nc
        qk, vaug = st
        pvsb = [[None] * NH, [None] * NH]

        def run_pass(which):
            if which == 2:
                pvt = self._attn_pass(qk[0][2], qk[1][2],
                                      [(0, 130), (1, 130)], vaug)
                for si in range(2):
                    dst = self.pvp.tile([65, T], BF16, tag=f"pv{si}2",
                                        name=f"pv{si}2")
                    nc.scalar.copy(dst[:], pvt[si][0:65, 0:T])
                    pvsb[si][2] = dst
            else:
                si = which
                pvt = self._attn_pass(qk[0][si], qk[1][si],
                                      [(si, 0), (si, 65)], vaug)
                for hh in range(2):
                    dst = self.pvp.tile([65, T], BF16, tag=f"pv{si}{hh}",
                                        name=f"pv{si}{hh}")
                    nc.scalar.copy(dst[:], pvt[hh][0:65, 0:T])
                    pvsb[si][hh] = dst

        # order: A-heads01, h2(A+B), proj A, B-heads01, proj B — proj A's
        # DVE work overlaps pass B on the PE.
        run_pass(0)
        run_pass(2)
        self._proj(2 * pr, 0, pvsb)
        run_pass(1)
        self._proj(2 * pr + 1, 1, pvsb)

    def _proj(self, s, si, pvsb):
        # fused projection + softmax normalize + bias
        # psum layout per (s, lc): h0@0 h1@256 h2@512 (Z at 192/448/704)
        nc = self.nc
        if True:
            obuf = self.op_.tile([128, 6 * CO], F32, tag=f"ob{si}", name=f"ob{si}")
            otl = self.op_.tile([17, CO], F32, tag=f"ot{si}", name=f"ot{si}")
            for lc, (l0, ln) in enumerate(TBLK):
                ps = self.psp.tile([128, 1024], F32,
                                   tag=("mm" if lc % 2 == 0 else "pv"),
                                   name="mm")
                for h in range(NH):
                    nc.tensor.matmul(
                        ps[0:ln, 256 * h:256 * h + CO + 1],
                        pvsb[si][h][:, l0:l0 + ln], self.wpa[h][:],
                        start=(h != 1), stop=(h != 0),
                        skip_group_check=True)
                r = self.rp.tile([128, 3], F32, tag="r", name="r")
                nc.vector.reciprocal(
                    r[0:ln, :].rearrange("p (h x) -> p h x", h=3, x=1),
                    ps[0:ln, 0:768].rearrange(
                        "p (h x) -> p h x", h=3, x=256)[:, :, CO:CO + 1])
                tmp = self.tmpp.tile([128, CO], F32, tag="t", name="t")
                nc.vector.scalar_tensor_tensor(
                    tmp[0:ln, :], ps[0:ln, 0:CO], r[0:ln, 0:1],
                    self.btile[0:ln, :], OP.mult, OP.add)
                nc.vector.scalar_tensor_tensor(
                    tmp[0:ln, :], ps[0:ln, 256:256 + CO], r[0:ln, 1:2],
                    tmp[0:ln, :], OP.mult, OP.add)
                dst = obuf[:, lc * CO:(lc + 1) * CO] if lc < 6 else otl[:]
                nc.vector.scalar_tensor_tensor(
                    dst[0:ln, :], ps[0:ln, 512:512 + CO], r[0:ln, 2:3],
                    tmp[0:ln, :], OP.mult, OP.add)
            nc.sync.dma_start(
                self.out_d[s, 0:768, :].rearrange("(n p) c -> p n c", p=128),
                obuf[:].rearrange("p (n c) -> p n c", n=6, c=CO))
            nc.sync.dma_start(self.out_d[s, 768:785, :], otl[:])


_NC_CACHE = None


def _flat_parity(xi, k):
    """xi [n, 784] -> [n, FLN] flat padded image at offset k."""
    n = xi.shape[0]
    p = np.zeros((n, FLN), dtype=ml_dtypes.bfloat16)
    p[:, k:k + 784] = xi
    return p


def _prep_host(inputs):
    x = np.asarray(inputs["x"], dtype=np.float32)
    conv_w = np.asarray(inputs["conv_w"], dtype=np.float32)   # [3,C,1,3,3]
    bn_scale = np.asarray(inputs["bn_scale"], dtype=np.float32)
    bn_bias = np.asarray(inputs["bn_bias"], dtype=np.float32)
    bn_mean = np.asarray(inputs["bn_mean"], dtype=np.float32)
    bn_var = np.asarray(inputs["bn_var"], dtype=np.float32)
    w_qkv = np.asarray(inputs["w_qkv"], dtype=np.float32)     # [3,CO,C]
    w_proj = np.asarray(inputs["w_proj"], dtype=np.float32)   # [CO,CO]
    b_proj = np.asarray(inputs["b_proj"], dtype=np.float32)   # [CO]

    xt = x.transpose(0, 2, 1).astype(ml_dtypes.bfloat16)       # [B, C, T]
    xcls = xt[:, :, 0]                                         # [B, C]
    ximg = xt[:, :, 1:]                                        # [B, C, 784]
    fa = np.zeros((B, C, FLN), dtype=ml_dtypes.bfloat16)
    fb = np.zeros((B, C, FLN), dtype=ml_dtypes.bfloat16)
    for b in range(B):
        fa[b] = _flat_parity(ximg[b], KA)
        fb[b] = _flat_parity(ximg[b], KB)
    fa[:, :, CLSPOS] = xcls
    # chunk0 pair-merged on free dim: [NP, 128, 2*FLN] = A | B
    xfa0 = np.concatenate([fa[0::2, 0:128], fa[1::2, 0:128]], axis=2)
    xfb0 = np.concatenate([fb[0::2, 0:128], fb[1::2, 0:128]], axis=2)
    # chunk1: A ch128.. on partitions 0-63, B on 64-127
    xfa1 = np.concatenate([fa[0::2, 128:192], fa[1::2, 128:192]], axis=1)
    xfb1 = np.concatenate([fb[0::2, 128:192], fb[1::2, 128:192]], axis=1)

    # BN fold into taps
    s = bn_scale / np.sqrt(bn_var + BN_EPS)                    # [3,C]
    wtap = conv_w[:, :, 0, :, :].reshape(3, C, 9) * s[:, :, None]
    bterm = bn_bias - bn_mean * s                               # [3,C]
    wc_full = np.ascontiguousarray(wtap.transpose(1, 0, 2).reshape(C, 27))
    bnt_full = np.ascontiguousarray(bterm.T)                   # [C,3]
    dup = lambda a: np.concatenate([a[128:192], a[128:192]], 0)
    wc_h = np.stack([wc_full[0:128], dup(wc_full)]).astype(np.float32)
    bnt_h = np.stack([bnt_full[0:128], dup(bnt_full)]).astype(np.float32)

    # wrap-garbage fix columns: fix[i, b, c, y, 0] = sum_dy w*img[y+dy-2, 27]
    # (left, x=0), [..., 1] = sum_dy w*img[y+dy, 0] (right, x=27)
    img3 = ximg.astype(np.float32).reshape(B, C, 28, 28)
    fix = np.zeros((3, B, C, 28, 2), dtype=np.float32)
    for i in range(3):
        for dy in range(3):
            wl = wtap[i, :, dy * 3]
            wr = wtap[i, :, dy * 3 + 2]
            for y in range(28):
                r = y + dy - 2
                if 0 <= r < 28:
                    fix[i, :, :, y, 0] += wl[None, :] * img3[:, :, r, 27]
                r2 = y + dy
                if 0 <= r2 < 28:
                    fix[i, :, :, y, 1] += wr[None, :] * img3[:, :, r2, 0]
    fix = fix.astype(ml_dtypes.bfloat16)
    fixf = fix.reshape(3, B, C, 56)
    NP = B // 2
    # fx0 [NP, 128, 3*112]: per conv i: A fixes (56) then B fixes (56)
    fx0 = np.concatenate([fixf[:, 0::2, 0:128], fixf[:, 1::2, 0:128]],
                         axis=3)                      # [3, NP, 128, 112]
    fx0 = np.ascontiguousarray(
        fx0.transpose(1, 2, 0, 3).reshape(NP, 128, 3 * 112))
    # fx1 [NP, 128, 3*56]: chunk1, A rows 0-63 / B rows 64-127
    fx1 = np.concatenate([fixf[:, 0::2, 128:192], fixf[:, 1::2, 128:192]],
                         axis=2)                      # [3, NP, 128, 56]
    fx1 = np.ascontiguousarray(
        fx1.transpose(1, 2, 0, 3).reshape(NP, 128, 3 * 56))

    # q/k projection weights as lhsT [c, o]; fold softmax scale into q
    wq = w_qkv[0].T * SCALE                                    # [C, CO]
    wk = w_qkv[1].T
    wv = w_qkv[2].T
    wqk1_h = np.stack([wq[0:128], wk[0:128]]).astype(ml_dtypes.bfloat16)
    wqk2_h = np.stack([dup(wq), dup(wk)]).astype(ml_dtypes.bfloat16)
    wv1_h = wv[0:128].astype(ml_dtypes.bfloat16)
    wv2_h = dup(wv).astype(ml_dtypes.bfloat16)

    # per-head output projection rhs [65, 193]: rows 0-63 = Wp_h^T, row 64 =
    # one-hot at col 192 (emits Z token-major)
    wpa_h = np.zeros((NH, 65, CO + 1), dtype=ml_dtypes.bfloat16)
    for h in range(NH):
        wpa_h[h, 0:64, 0:CO] = w_proj[:, h * 64:(h + 1) * 64].T.astype(
            ml_dtypes.bfloat16)
        wpa_h[h, 64, CO] = 1.0
    btile_h = np.ascontiguousarray(
        np.broadcast_to(b_proj[None, :], (128, CO))).astype(np.float32)

    return (xfa0, xfb0, xfa1, xfb1, fx0, fx1, wqk1_h, wqk2_h, wv1_h, wv2_h,
            wpa_h, wc_h, bnt_h, btile_h)


def kernel(**inputs):
    global _NC_CACHE
    (xfa0, xfb0, xfa1, xfb1, fx0, fx1, wqk1_h, wqk2_h, wv1_h, wv2_h,
     wpa_h, wc_h, bnt_h, btile_h) = _prep_host(inputs)

    if _NC_CACHE is None:
        _NC_CACHE = build_bass()
    nc = _NC_CACHE

    PPC = NPAIR  # pairs per core
    sh = lambda a: a.reshape(NCORES, PPC, *a.shape[1:])
    xfa0, xfb0, xfa1, xfb1 = sh(xfa0), sh(xfb0), sh(xfa1), sh(xfb1)
    fx0, fx1 = sh(fx0), sh(fx1)
    in_maps = [
        {"xfa0": np.ascontiguousarray(xfa0[c]),
         "xfb0": np.ascontiguousarray(xfb0[c]),
         "xfa1": np.ascontiguousarray(xfa1[c]),
         "xfb1": np.ascontiguousarray(xfb1[c]),
         "fx0": np.ascontiguousarray(fx0[c]),
         "fx1": np.ascontiguousarray(fx1[c]),
         "wqk1": wqk1_h, "wqk2": wqk2_h, "wv1": wv1_h, "wv2": wv2_h,
         "wpa": wpa_h, "wc": wc_h, "bnt": bnt_h,
         "btile": btile_h}
        for c in range(NCORES)
    ]
    res = run_bass_kernel_spmd(nc, in_maps, list(range(NCORES)), **RUN_KWARGS)
    global LAST_RESULTS
    LAST_RESULTS = res
    out = np.concatenate([np.asarray(r["out"]) for r in res.results], axis=0)
    return out.reshape(B, T, CO).astype(np.float32)


RUN_KWARGS = {}
LAST_RESULTS = None


# revision 48
# speedup vs baseline: 1.1071x; 1.0027x over previous
"""Trainium2 Bass kernel for nn_Attention_51634096833229 (v2.1).

CvT-style conv-projection attention: depthwise 3x3 conv + BN on the 28x28
token image for q/k/v, linear qkv projections, 3-head attention over 785
tokens, output projection.  Data-parallel over batch: B=32 -> 4 samples
(2 sample-pairs) per core on 8 cores.

Design:
  - host supplies PRE-PADDED bf16 images in both alignment parities
    (image at odd / even column offset of 32-wide padded rows), pair-merged:
    chunk0 = channels 0..127 of samples A|B side by side on the free dim,
    chunk1 = channels 128..191 of A (partitions 0-63) and B (64-127).
    cls token stashed at never-read pad position [row 0, col 30].
    -> zero on-device layout prep; every conv tap runs in the DVE 2x mode.
  - depthwise conv + BN entirely on DVE: 27 scalar_tensor_tensor taps per
    chunk-tile, chunk0 processing both samples in one op (FD=1568).
  - K=64 matmuls issued as concurrent row/col-tiled pairs (tile_position
    derived from base partitions): head0+head1 scores, cross-sample head2,
    half-K projection chunks.
  - softmax scale folded into w_q host-side; exp on ACT psum->sbuf bf16;
    psum evacuation copies (qk, PV, v-scatter) on ACT.
  - softmax normalization fused into the output projection: per-head proj
    with K=65 (P^T rows + Z row), one-hot rhs column emits Z token-major at
    psum cols {192,448,704}; one packed DVE reciprocal; 3 scalar_tensor_
    tensor ops combine heads with per-partition 1/Z plus a bias tile.
"""

import sys

sys.path.insert(0, "/opt/trn_rl_repo")

import numpy as np
import ml_dtypes

import concourse.bass as bass
import concourse.mybir as mybir
import concourse.tile as tile
from concourse import bacc
from concourse.bass_utils import run_bass_kernel_spmd

F32 = mybir.dt.float32
BF16 = mybir.dt.bfloat16
AF = mybir.ActivationFunctionType
OP = mybir.AluOpType

B, T, C, CO, NH, D = 32, 785, 192, 192, 3, 64
NCORES = 8
BPC = B // NCORES          # samples per core
NPAIR = BPC // 2           # sample pairs per core
SCALE = float(CO) ** -0.5
BN_EPS = 1e-5
TC = 786                   # y columns: [dummy, cls, img x 784]
FLN = 844                  # flat padded image length (2B-parity copies)
KA = 29                    # image base offset in flat copy A (taps dx in {0,2})
KB = 30                    # image base offset in flat copy B (taps dx == 1)
CLSPOS = 842               # cls position in flat copy A (never read by taps)
NSEG = [(0, 512), (512, T - 512)]
TBLK = [(i * 128, min(128, T - i * 128)) for i in range((T + 127) // 128)]


def build_bass():
    return Kern().build()


class Kern:
    def __init__(self):
        nc = bacc.Bacc(None)
        self.nc = nc
        dd = nc.declare_dram_parameter
        self.xfa0_d = dd("xfa0", [NPAIR, 128, 2 * FLN], BF16, isOutput=False)
        self.xfb0_d = dd("xfb0", [NPAIR, 128, 2 * FLN], BF16, isOutput=False)
        self.xfa1_d = dd("xfa1", [NPAIR, 128, FLN], BF16, isOutput=False)
        self.xfb1_d = dd("xfb1", [NPAIR, 128, FLN], BF16, isOutput=False)
        self.fx0_d = dd("fx0", [NPAIR, 128, 3 * 112], BF16, isOutput=False)
        self.fx1_d = dd("fx1", [NPAIR, 128, 3 * 56], BF16, isOutput=False)
        self.wqk1_d = dd("wqk1", [2, 128, CO], BF16, isOutput=False)
        self.wqk2_d = dd("wqk2", [2, 128, CO], BF16, isOutput=False)
        self.wv1_d = dd("wv1", [128, CO], BF16, isOutput=False)
        self.wv2_d = dd("wv2", [128, CO], BF16, isOutput=False)
        self.wpa_d = dd("wpa", [NH, 65, CO + 1], BF16, isOutput=False)
        self.wc_d = dd("wc", [2, 128, 27], F32, isOutput=False)
        self.bnt_d = dd("bnt", [2, 128, 3], F32, isOutput=False)
        self.bt_d = dd("btile", [128, CO], F32, isOutput=False)
        self.out_d = dd("out", [BPC, T, CO], F32, isOutput=True)

    def build(self):
        nc = self.nc
        from contextlib import ExitStack
        with tile.TileContext(nc) as tc, ExitStack() as es:
            self.consts = es.enter_context(tc.tile_pool(name="consts", bufs=1))
            self.psp = es.enter_context(tc.tile_pool(name="ps", bufs=2, space="PSUM"))
            self.padp = es.enter_context(tc.tile_pool(name="pad", bufs=2))
            self.yp = es.enter_context(tc.tile_pool(name="y", bufs=2))
            self.qkp = es.enter_context(tc.tile_pool(name="qk", bufs=2))
            self.ep = es.enter_context(tc.tile_pool(name="E", bufs=3))
            self.pvp = es.enter_context(tc.tile_pool(name="pv", bufs=2))
            self.rp = es.enter_context(tc.tile_pool(name="r", bufs=3))
            self.tmpp = es.enter_context(tc.tile_pool(name="tmp", bufs=3))
            self.op_ = es.enter_context(tc.tile_pool(name="osb", bufs=2))
            self.vap = es.enter_context(tc.tile_pool(name="vaug", bufs=2))
            self._consts()
            pads = [self._load(pr) for pr in range(NPAIR)]
            # software pipeline: pair p's conv chains (DVE) are emitted
            # before pair p-1's attention so they overlap on different
            # engines; pair p's projection MMs follow the attention.
            st, _ = self._conv_qkv(0, pads[0], interleave=True)
            for pr in range(1, NPAIR):
                _, ycs = self._conv_qkv(pr, pads[pr], interleave=False)
                self._attn_proj(pr - 1, st)
                st = self._qkv_mms(ycs)
            self._attn_proj(NPAIR - 1, st)
        if not nc.is_finalized():
            nc.finalize()
        return nc

    def _consts(self):
        nc, consts = self.nc, self.consts
        self.wqk1, self.wqk2 = [], []
        for i in range(2):
            t1 = consts.tile([128, CO], BF16, tag=f"wqk1{i}", name=f"wqk1{i}")
            nc.sync.dma_start(t1[:], self.wqk1_d[i])
            self.wqk1.append(t1)
            t2 = consts.tile([128, CO], BF16, tag=f"wqk2{i}", name=f"wqk2{i}")
            nc.sync.dma_start(t2[:], self.wqk2_d[i])
            self.wqk2.append(t2)
        self.wv1 = consts.tile([128, CO], BF16, tag="wv1", name="wv1")
        nc.sync.dma_start(self.wv1[:], self.wv1_d[:])
        self.wv2 = consts.tile([128, CO], BF16, tag="wv2", name="wv2")
        nc.sync.dma_start(self.wv2[:], self.wv2_d[:])
        self.wpa = []
        for h in range(NH):
            t = consts.tile([65, CO + 1], BF16, tag=f"wpa{h}", name=f"wpa{h}")
            nc.sync.dma_start(t[:], self.wpa_d[h])
            self.wpa.append(t)
        self.wc, self.bnt = [], []
        for ci in range(2):
            t = consts.tile([128, 27], F32, tag=f"wc{ci}", name=f"wc{ci}")
            nc.sync.dma_start(t[:], self.wc_d[ci])
            self.wc.append(t)
            t2 = consts.tile([128, 3], F32, tag=f"bnt{ci}", name=f"bnt{ci}")
            nc.sync.dma_start(t2[:], self.bnt_d[ci])
            self.bnt.append(t2)
        self.btile = consts.tile([128, CO], F32, tag="btile", name="btile")
        nc.sync.dma_start(self.btile[:], self.bt_d[:])

        # HAM warm-up: ~6us of dummy matmuls during the conv head so the PE
        # clock gate opens (K=8/8, 2.4 GHz) before the real matmul stream.
        warm = self.psp.tile([128, 1024], F32, tag="mm", name="warm")
        for _ in range(20):
            nc.tensor.matmul(warm[0:128, 0:CO], self.wqk1[0][:, 0:128],
                             self.wqk1[1][:], start=True, stop=True)

    def _load(self, pr):
        nc = self.nc
        fa0 = self.padp.tile([128, 2 * FLN], BF16, tag="fa0", name="fa0")
        nc.sync.dma_start(fa0[:], self.xfa0_d[pr])
        fb0 = self.padp.tile([128, 2 * FLN], BF16, tag="fb0", name="fb0")
        nc.sync.dma_start(fb0[:], self.xfb0_d[pr])
        fa1 = self.padp.tile([128, FLN], BF16, tag="fa1", name="fa1")
        nc.sync.dma_start(fa1[:], self.xfa1_d[pr])
        fb1 = self.padp.tile([128, FLN], BF16, tag="fb1", name="fb1")
        nc.sync.dma_start(fb1[:], self.xfb1_d[pr])
        fx0 = self.padp.tile([128, 3 * 112], BF16, tag="fx0", name="fx0")
        nc.sync.dma_start(fx0[:], self.fx0_d[pr])
        fx1 = self.padp.tile([128, 3 * 56], BF16, tag="fx1", name="fx1")
        nc.sync.dma_start(fx1[:], self.fx1_d[pr])
        return (fa0, fb0, fa1, fb1, fx0, fx1)

    def _conv_chain(self, i, j, pads):
        """Depthwise conv i -> y bf16 via flat-1D taps + one fix-column TT.
        j=0: chunk0 of A and B pair-merged on free dim -> y [128, 2*TC];
        j=1: chunk1 (A rows 0-63, B 64-127) -> y [128, TC]."""
        nc = self.nc
        fa0, fb0, fa1, fb1, fx0, fx1 = pads
        if j == 0:
            fa, fb, fx, ci, ns = fa0, fb0, fx0, 0, 2
        else:
            fa, fb, fx, ci, ns = fa1, fb1, fx1, 1, 1
        y = self.yp.tile([128, ns * TC], BF16, tag=f"y{i}{j}", name=f"y{i}{j}")
        yv = y.rearrange("p (s c) -> p s c", s=ns, c=TC)
        yf = yv[:, :, 2:TC]
        fav = fa.rearrange("p (s c) -> p s c", s=ns, c=FLN)
        fbv = fb.rearrange("p (s c) -> p s c", s=ns, c=FLN)
        # cls column first (independent of the tap chain)
        nc.vector.tensor_copy(yv[:, :, 1:2], fav[:, :, CLSPOS:CLSPOS + 1])

        def tapsrc(tap):
            dy, dx = tap // 3, tap % 3
            if dx == 1:
                return fbv[:, :, 2 + 28 * dy:2 + 28 * dy + 784]
            return fav[:, :, 28 * dy + dx:28 * dy + dx + 784]

        if False:
            pass
        else:
            for tap in range(9):
                wcol = self.wc[ci][:, i * 9 + tap:i * 9 + tap + 1]
                if tap == 0:
                    nc.vector.tensor_scalar(yf, tapsrc(tap), wcol,
                                            self.bnt[ci][:, i:i + 1],
                                            OP.mult, OP.add)
                else:
                    nc.vector.scalar_tensor_tensor(yf, tapsrc(tap), wcol, yf,
                                                   OP.mult, OP.add)
        # one fix-column TT per sample slot: subtract host-computed wrap
        # garbage at image columns {0, 27}
        for s in range(ns):
            dst = yv[:, s, 2:TC].rearrange(
                "p (a b) -> p a b", a=28, b=28)[:, :, 0:28:27]
            fxs = fx[:, i * ns * 56 + s * 56:i * ns * 56 + (s + 1) * 56]
            nc.vector.tensor_tensor(
                dst, dst, fxs.rearrange("p (a b) -> p a b", a=28, b=2),
                OP.subtract)
        return y

    def _chains_conv(self, i, pads):
        return [self._conv_chain(i, 0, pads), self._conv_chain(i, 1, pads)]

    def _mms_qk(self, i, ysc):
        """Projection matmuls for conv i (q or k). ysc = [y0pair, y1].
        Returns the three qkT tiles [A-heads01, B-heads01, h2-pair]."""
        nc = self.nc
        y0p, y1 = ysc
        ys = [y0p[:, 0:TC], y0p[:, TC:2 * TC], y1]
        row = [None] * 3
        # chunk0 of A and B -> two live psum tiles; half-K matmuls of A
        # (rows 0:64) and B (rows 64:128) emitted adjacently -> concurrent
        pss = [self.psp.tile([128, 1024], F32, tag="mm", name=f"mmq{si}")
               for si in range(2)]
        for si in range(2):
            for (n0, nn) in NSEG:
                nc.tensor.matmul(
                    pss[si][0:128, n0:n0 + nn],
                    self.wqk1[i][:, 0:128],
                    ys[si][:, 1 + n0:1 + n0 + nn],
                    start=True, stop=False)
        for (n0, nn) in NSEG:
            for si in range(2):
                nc.tensor.matmul(
                    pss[si][0:128, n0:n0 + nn],
                    self.wqk2[i][si * 64:(si + 1) * 64, 0:128],
                    ys[2][si * 64:(si + 1) * 64, 1 + n0:1 + n0 + nn],
                    start=False, stop=True)
        for si in range(2):
            dst = self.qkp.tile([128, T], BF16, tag=f"qk{i}{si}",
                                name=f"qk{i}{si}")
            nc.scalar.copy(dst[:], pss[si][0:128, 0:T])
            row[si] = dst
        # head2 of A (tileA rows 0-63, col strips 0-1) and B (tileB rows
        # 64-127, col strips 2-3): col-concurrent, separate psum banks.
        ps2 = [self.psp.tile([128, 1024], F32, tag="mm", name=f"mmh{si}")
               for si in range(2)]
        for (n0, nn) in NSEG:
            nc.tensor.matmul(
                ps2[0][0:64, n0:n0 + nn],
                self.wqk1[i][:, 128:192],
                ys[0][:, 1 + n0:1 + n0 + nn],
                start=True, stop=False)
            nc.tensor.matmul(
                ps2[1][64:128, n0:n0 + nn],
                self.wqk1[i][:, 128:192],
                ys[1][:, 1 + n0:1 + n0 + nn],
                start=True, stop=False)
            nc.tensor.matmul(
                ps2[0][0:64, n0:n0 + nn],
                self.wqk2[i][0:64, 128:192], ys[2][0:64, 1 + n0:1 + n0 + nn],
                start=False, stop=True)
            nc.tensor.matmul(
                ps2[1][64:128, n0:n0 + nn],
                self.wqk2[i][64:128, 128:192], ys[2][64:128, 1 + n0:1 + n0 + nn],
                start=False, stop=True)
        dst = self.qkp.tile([128, T], BF16, tag=f"qk{i}2", name=f"qk{i}2")
        nc.scalar.copy(dst[0:64, :], ps2[0][0:64, 0:T])
        nc.scalar.copy(dst[64:128, :], ps2[1][64:128, 0:T])
        row[2] = dst
        return row

    def _mms_v(self, ysc):
        """v projection (token-major) + vaug scatter. ysc = [y0pair, y1]."""
        nc = self.nc
        y0p, y1 = ysc
        yv = [y0p[:, 0:TC], y0p[:, TC:2 * TC], y1]
        vaug = [
            [self.vap.tile([128, 3 * 65], BF16, tag=f"va{s}{tb}",
                           name=f"va{s}{tb}") for tb in range(len(TBLK))]
            for s in range(2)]
        for s in range(2):
            for tb, (t0, tn) in enumerate(TBLK):
                nc.vector.memset(
                    vaug[s][tb][0:tn, :].rearrange(
                        "p (h d) -> p h d", h=3, d=65)[:, :, 64:65], 1.0)
        for tbq in range(0, len(TBLK), 2):
            pss = [self.psp.tile([128, 1024], F32, tag="mm", name=f"mmv{si}")
                   for si in range(2)]
            for k in range(2):
                if tbq + k >= len(TBLK):
                    break
                t0, tn = TBLK[tbq + k]
                for si in range(2):
                    nc.tensor.matmul(
                        pss[si][0:tn, 512 * k:512 * k + CO],
                        yv[si][:, 1 + t0:1 + t0 + tn],
                        self.wv1[:], start=True, stop=False)
                for si in range(2):
                    nc.tensor.matmul(
                        pss[si][0:tn, 512 * k:512 * k + CO],
                        yv[2][si * 64:(si + 1) * 64, 1 + t0:1 + t0 + tn],
                        self.wv2[si * 64:(si + 1) * 64, :],
                        start=False, stop=True)
            for k in range(2):
                if tbq + k >= len(TBLK):
                    break
                t0, tn = TBLK[tbq + k]
                for si in range(2):
                    dst = vaug[si][tbq + k][0:tn, :].rearrange(
                        "p (h d) -> p h d", h=3, d=65)[:, :, 0:64]
                    nc.scalar.copy(
                        dst,
                        pss[si][0:tn, 512 * k:512 * k + CO].rearrange(
                            "p (h d) -> p h d", h=3, d=64))
        return vaug

    def _conv_qkv(self, pr, pads, interleave):
        """Emit conv chains and projection MMs for a pair.  When interleave
        is True, chains and MMs alternate per conv (fills the PE early);
        otherwise chains only — call _qkv_mms later with the returned state."""
        if interleave:
            qk = [None, None]
            ycs = [None] * 3
            for i in range(2):
                ycs[i] = self._chains_conv(i, pads)
                qk[i] = self._mms_qk(i, ycs[i])
            ycs[2] = self._chains_conv(2, pads)
            vaug = self._mms_v(ycs[2])
            return (qk, vaug), None
        ycs = [self._chains_conv(i, pads) for i in range(3)]
        return None, ycs

    def _qkv_mms(self, ycs):
        qk = [self._mms_qk(i, ycs[i]) for i in range(2)]
        vaug = self._mms_v(ycs[2])
        return qk, vaug

    def _attn_pass(self, lhs_q, lhs_k, va_sel, vaug):
        """One attention pass: two row-tiled units (hh=0 rows 0:64, hh=1 rows
        64:128 of lhs_q/lhs_k).  va_sel[hh] = (si, vaug col base).  Returns
        psum pv tiles [65, T] per hh (caller copies out)."""
        nc = self.nc
        pvt = [self.psp.tile([128, 1024], F32, tag="pv", name="pv")
               for _ in range(2)]
        nblk = len(TBLK)

        def emit_pv(tb, es_):
            t0, tn = TBLK[tb]
            for hh in range(2):
                si, cb = va_sel[hh]
                for (n0, nn) in NSEG:
                    nc.tensor.matmul(
                        pvt[hh][0:65, n0:n0 + nn],
                        vaug[si][tb][0:tn, cb:cb + 65],
                        es_[hh][0:tn, n0:n0 + nn],
                        start=(tb == 0), stop=(tb == nblk - 1))

        prev = None  # PV trails scores by one tb so it never waits on exp
        for tb, (t0, tn) in enumerate(TBLK):
            es_ = []
            for hh in range(2):
                kh = lhs_k[hh * 64:(hh + 1) * 64, :]
                qh = lhs_q[hh * 64:(hh + 1) * 64, :]
                ss = self.psp.tile([128, 1024], F32, tag="mm", name="mm")
                for (n0, nn) in NSEG:
                    nc.tensor.matmul(
                        ss[0:tn, n0:n0 + nn], kh[:, t0:t0 + tn],
                        qh[:, n0:n0 + nn], start=True, stop=True)
                e = self.ep.tile([128, T], BF16, tag="E", name="E")
                nc.scalar.activation(e[0:tn, 0:T], ss[0:tn, 0:T], AF.Exp)
                es_.append(e)
            if prev is not None:
                emit_pv(tb - 1, prev)
            prev = es_
        emit_pv(nblk - 1, prev)
        return pvt

    def _attn_proj(self, pr, st):
        nc = self.nc
        qk, vaug = st
        pvsb = [[None] * NH, [None] * NH]

        def run_pass(which):
            if which == 2:
                pvt = self._attn_pass(qk[0][2], qk[1][2],
                                      [(0, 130), (1, 130)], vaug)
                for si in range(2):
                    dst = self.pvp.tile([65, T], BF16, tag=f"pv{si}2",
                                        name=f"pv{si}2")
                    nc.scalar.copy(dst[:], pvt[si][0:65, 0:T])
                    pvsb[si][2] = dst
            else:
                si = which
                pvt = self._attn_pass(qk[0][si], qk[1][si],
                                      [(si, 0), (si, 65)], vaug)
                for hh in range(2):
                    dst = self.pvp.tile([65, T], BF16, tag=f"pv{si}{hh}",
                                        name=f"pv{si}{hh}")
                    nc.scalar.copy(dst[:], pvt[hh][0:65, 0:T])
                    pvsb[si][hh] = dst

        # order: A-heads01, h2(A+B), proj A, B-heads01, proj B — proj A's
        # DVE work overlaps pass B on the PE.
        run_pass(0)
        run_pass(2)
        self._proj(2 * pr, 0, pvsb)
        run_pass(1)
        self._proj(2 * pr + 1, 1, pvsb)

    def _proj(self, s, si, pvsb):
        # fused projection + softmax normalize + bias
        # psum layout per (s, lc): h0@0 h1@256 h2@512 (Z at 192/448/704)
        nc = self.nc
        if True:
            obuf = self.op_.tile([128, 6 * CO], F32, tag=f"ob{si}", name=f"ob{si}")
            otl = self.op_.tile([17, CO], F32, tag=f"ot{si}", name=f"ot{si}")
            for lc, (l0, ln) in enumerate(TBLK):
                ps = self.psp.tile([128, 1024], F32,
                                   tag=("mm" if lc % 2 == 0 else "pv"),
                                   name="mm")
                for h in range(NH):
                    nc.tensor.matmul(
                        ps[0:ln, 256 * h:256 * h + CO + 1],
                        pvsb[si][h][:, l0:l0 + ln], self.wpa[h][:],
                        start=(h != 1), stop=(h != 0),
                        skip_group_check=True)
                r = self.rp.tile([128, 3], F32, tag="r", name="r")
                nc.vector.reciprocal(
                    r[0:ln, :].rearrange("p (h x) -> p h x", h=3, x=1),
                    ps[0:ln, 0:768].rearrange(
                        "p (h x) -> p h x", h=3, x=256)[:, :, CO:CO + 1])
                tmp = self.tmpp.tile([128, CO], F32, tag="t", name="t")
                nc.vector.scalar_tensor_tensor(
                    tmp[0:ln, :], ps[0:ln, 0:CO], r[0:ln, 0:1],
                    self.btile[0:ln, :], OP.mult, OP.add)
                nc.vector.scalar_tensor_tensor(
                    tmp[0:ln, :], ps[0:ln, 256:256 + CO], r[0:ln, 1:2],
                    tmp[0:ln, :], OP.mult, OP.add)
                dst = obuf[:, lc * CO:(lc + 1) * CO] if lc < 6 else otl[:]
                nc.vector.scalar_tensor_tensor(
                    dst[0:ln, :], ps[0:ln, 512:512 + CO], r[0:ln, 2:3],
                    tmp[0:ln, :], OP.mult, OP.add)
            nc.sync.dma_start(
                self.out_d[s, 0:768, :].rearrange("(n p) c -> p n c", p=128),
                obuf[:].rearrange("p (n c) -> p n c", n=6, c=CO))
            nc.sync.dma_start(self.out_d[s, 768:785, :], otl[:])


_NC_CACHE = None


def _flat_parity(xi, k):
    """xi [n, 784] -> [n, FLN] flat padded image at offset k."""
    n = xi.shape[0]
    p = np.zeros((n, FLN), dtype=ml_dtypes.bfloat16)
    p[:, k:k + 784] = xi
    return p


def _prep_host(inputs):
    x = np.asarray(inputs["x"], dtype=np.float32)
    conv_w = np.asarray(inputs["conv_w"], dtype=np.float32)   # [3,C,1,3,3]
    bn_scale = np.asarray(inputs["bn_scale"], dtype=np.float32)
    bn_bias = np.asarray(inputs["bn_bias"], dtype=np.float32)
    bn_mean = np.asarray(inputs["bn_mean"], dtype=np.float32)
    bn_var = np.asarray(inputs["bn_var"], dtype=np.float32)
    w_qkv = np.asarray(inputs["w_qkv"], dtype=np.float32)     # [3,CO,C]
    w_proj = np.asarray(inputs["w_proj"], dtype=np.float32)   # [CO,CO]
    b_proj = np.asarray(inputs["b_proj"], dtype=np.float32)   # [CO]

    xt = x.transpose(0, 2, 1).astype(ml_dtypes.bfloat16)       # [B, C, T]
    xcls = xt[:, :, 0]                                         # [B, C]
    ximg = xt[:, :, 1:]                                        # [B, C, 784]
    fa = np.zeros((B, C, FLN), dtype=ml_dtypes.bfloat16)
    fb = np.zeros((B, C, FLN), dtype=ml_dtypes.bfloat16)
    for b in range(B):
        fa[b] = _flat_parity(ximg[b], KA)
        fb[b] = _flat_parity(ximg[b], KB)
    fa[:, :, CLSPOS] = xcls
    # chunk0 pair-merged on free dim: [NP, 128, 2*FLN] = A | B
    xfa0 = np.concatenate([fa[0::2, 0:128], fa[1::2, 0:128]], axis=2)
    xfb0 = np.concatenate([fb[0::2, 0:128], fb[1::2, 0:128]], axis=2)
    # chunk1: A ch128.. on partitions 0-63, B on 64-127
    xfa1 = np.concatenate([fa[0::2, 128:192], fa[1::2, 128:192]], axis=1)
    xfb1 = np.concatenate([fb[0::2, 128:192], fb[1::2, 128:192]], axis=1)

    # BN fold into taps
    s = bn_scale / np.sqrt(bn_var + BN_EPS)                    # [3,C]
    wtap = conv_w[:, :, 0, :, :].reshape(3, C, 9) * s[:, :, None]
    bterm = bn_bias - bn_mean * s                               # [3,C]
    wc_full = np.ascontiguousarray(wtap.transpose(1, 0, 2).reshape(C, 27))
    bnt_full = np.ascontiguousarray(bterm.T)                   # [C,3]
    dup = lambda a: np.concatenate([a[128:192], a[128:192]], 0)
    wc_h = np.stack([wc_full[0:128], dup(wc_full)]).astype(np.float32)
    bnt_h = np.stack([bnt_full[0:128], dup(bnt_full)]).astype(np.float32)

    # wrap-garbage fix columns: fix[i, b, c, y, 0] = sum_dy w*img[y+dy-2, 27]
    # (left, x=0), [..., 1] = sum_dy w*img[y+dy, 0] (right, x=27)
    img3 = ximg.astype(np.float32).reshape(B, C, 28, 28)
    fix = np.zeros((3, B, C, 28, 2), dtype=np.float32)
    for i in range(3):
        for dy in range(3):
            wl = wtap[i, :, dy * 3]
            wr = wtap[i, :, dy * 3 + 2]
            for y in range(28):
                r = y + dy - 2
                if 0 <= r < 28:
                    fix[i, :, :, y, 0] += wl[None, :] * img3[:, :, r, 27]
                r2 = y + dy
                if 0 <= r2 < 28:
                    fix[i, :, :, y, 1] += wr[None, :] * img3[:, :, r2, 0]
    fix = fix.astype(ml_dtypes.bfloat16)
    fixf = fix.reshape(3, B, C, 56)
    NP = B // 2
    # fx0 [NP, 128, 3*112]: per conv i: A fixes (56) then B fixes (56)
    fx0 = np.concatenate([fixf[:, 0::2, 0:128], fixf[:, 1::2, 0:128]],
                         axis=3)                      # [3, NP, 128, 112]
    fx0 = np.ascontiguousarray(
        fx0.transpose(1, 2, 0, 3).reshape(NP, 128, 3 * 112))
    # fx1 [NP, 128, 3*56]: chunk1, A rows 0-63 / B rows 64-127
    fx1 = np.concatenate([fixf[:, 0::2, 128:192], fixf[:, 1::2, 128:192]],
                         axis=2)                      # [3, NP, 128, 56]
    fx1 = np.ascontiguousarray(
        fx1.transpose(1, 2, 0, 3).reshape(NP, 128, 3 * 56))

    # q/k projection weights as lhsT [c, o]; fold softmax scale into q
    wq = w_qkv[0].T * SCALE                                    # [C, CO]
    wk = w_qkv[1].T
    wv = w_qkv[2].T
    wqk1_h = np.stack([wq[0:128], wk[0:128]]).astype(ml_dtypes.bfloat16)
    wqk2_h = np.stack([dup(wq), dup(wk)]).astype(ml_dtypes.bfloat16)
    wv1_h = wv[0:128].astype(ml_dtypes.bfloat16)
    wv2_h = dup(wv).astype(ml_dtypes.bfloat16)

    # per-head output projection rhs [65, 193]: rows 0-63 = Wp_h^T, row 64 =
    # one-hot at col 192 (emits Z token-major)
    wpa_h = np.zeros((NH, 65, CO + 1), dtype=ml_dtypes.bfloat16)
    for h in range(NH):
        wpa_h[h, 0:64, 0:CO] = w_proj[:, h * 64:(h + 1) * 64].T.astype(
            ml_dtypes.bfloat16)
        wpa_h[h, 64, CO] = 1.0
    btile_h = np.ascontiguousarray(
        np.broadcast_to(b_proj[None, :], (128, CO))).astype(np.float32)

    return (xfa0, xfb0, xfa1, xfb1, fx0, fx1, wqk1_h, wqk2_h, wv1_h, wv2_h,
            wpa_h, wc_h, bnt_h, btile_h)


def kernel(**inputs):
    global _NC_CACHE
    (xfa0, xfb0, xfa1, xfb1, fx0, fx1, wqk1_h, wqk2_h, wv1_h, wv2_h,
     wpa_h, wc_h, bnt_h, btile_h) = _prep_host(inputs)

    if _NC_CACHE is None:
        _NC_CACHE = build_bass()
    nc = _NC_CACHE

    PPC = NPAIR  # pairs per core
    sh = lambda a: a.reshape(NCORES, PPC, *a.shape[1:])
    xfa0, xfb0, xfa1, xfb1 = sh(xfa0), sh(xfb0), sh(xfa1), sh(xfb1)
    fx0, fx1 = sh(fx0), sh(fx1)
    in_maps = [
        {"xfa0": np.ascontiguousarray(xfa0[c]),
         "xfb0": np.ascontiguousarray(xfb0[c]),
         "xfa1": np.ascontiguousarray(xfa1[c]),
         "xfb1": np.ascontiguousarray(xfb1[c]),
         "fx0": np.ascontiguousarray(fx0[c]),
         "fx1": np.ascontiguousarray(fx1[c]),
         "wqk1": wqk1_h, "wqk2": wqk2_h, "wv1": wv1_h, "wv2": wv2_h,
         "wpa": wpa_h, "wc": wc_h, "bnt": bnt_h,
         "btile": btile_h}
        for c in range(NCORES)
    ]
    res = run_bass_kernel_spmd(nc, in_maps, list(range(NCORES)), **RUN_KWARGS)
    global LAST_RESULTS
    LAST_RESULTS = res
    out = np.concatenate([np.asarray(r["out"]) for r in res.results], axis=0)
    return out.reshape(B, T, CO).astype(np.float32)


RUN_KWARGS = {}
LAST_RESULTS = None
